# revision 7
# baseline (speedup 1.0000x reference)
"""Multi-head attention forward on 8 Trainium2 NeuronCores.

Problem: x [2,2048,1024], weights wq/wk/wv/wo [1024,1024] (torch Linear
layout, y = x @ W.T), 16 heads, head_dim 64, fp32.

Sharding: core c handles batch b = c//4 and head group g = c%4 (heads
4g..4g+3, i.e. 256 output dims of wq/wk/wv and 256 input dims of wo).
Each core computes a partial output [2048, 1024]; the host sums the 4
partials per batch.

On-core plan (all matmuls in float32r — fp32 data, PE rounds to ~13-bit
mantissa, 1 cycle/row at free-dim >= 256):
  qT, kT [256, 2048] = W_g @ x^T          (x^T supplied by host)
  v      [2048, 256] = x @ W_g^T, stored per s-tile with a ones column
                       appended per head (v_aug [128, 65] tiles)
  per (head, i-block of 1024):
    per j-tile (128 keys): scoresT [128 j, 1024 i] = kT_j^T @ qT
        pT = exp(0.125 * scoresT)               (ACT, writes f32r)
        o_aug [65, 1024] += v_aug_j^T @ pT      (row 64 = colsum)
    recip = 1/colsum; broadcast across partitions via PE outer product
    o_sb[h] = o_aug[0:64] * recip_bcast         (normalized o^T, f32r)
  out[i-tile, :] = sum_h o_sb[h]^T @ woT_h      (PSUM-accumulated)
"""

import numpy as np
from contextlib import ExitStack

import concourse.bacc as bacc
import concourse.bass as bass
import concourse.mybir as mybir
import concourse.tile as tile
from concourse.bass_utils import run_bass_kernel_spmd

f32 = mybir.dt.float32
f32r = mybir.dt.float32r
EXP = mybir.ActivationFunctionType.Exp

B, S, D = 2, 2048, 1024
H, DH = 16, 64
NCORES = 8
GROUPS = NCORES // B           # 4 head-groups per batch
HPC = H // GROUPS              # 4 heads per core
DLOC = HPC * DH                # 256
KT = D // 128                  # 8 contraction tiles
ST = S // 128                  # 16 sequence tiles
NB = 2                         # i-blocks
IB = S // NB                   # 1024
NCH = IB // 512                # 512-wide matmul chunks per i-block


def _emit(tc, nc):
    xT = nc.dram_tensor("xT", [D, S], f32, kind="ExternalInput").ap()
    wqT = nc.dram_tensor("wqT", [D, DLOC], f32, kind="ExternalInput").ap()
    wkT = nc.dram_tensor("wkT", [D, DLOC], f32, kind="ExternalInput").ap()
    wvT = nc.dram_tensor("wvT", [D, DLOC], f32, kind="ExternalInput").ap()
    woT = nc.dram_tensor("woT", [DLOC, D], f32, kind="ExternalInput").ap()
    outp = nc.dram_tensor("outp", [S, D], f32, kind="ExternalOutput").ap()

    with ExitStack() as ctx:
        wpool = ctx.enter_context(tc.tile_pool(name="wpool", bufs=1))
        qkv = ctx.enter_context(tc.tile_pool(name="qkv", bufs=1))
        small = ctx.enter_context(tc.tile_pool(name="small", bufs=2))
        ps = ctx.enter_context(tc.tile_pool(name="ps", bufs=3, space="PSUM"))
        pso = ctx.enter_context(tc.tile_pool(name="pso", bufs=1, space="PSUM"))

        # ---- constants ----
        ones_f = small.tile([128, HPC], f32, bufs=1)
        nc.vector.memset(ones_f, 1.0)
        ones65f = small.tile([65, 64], f32, bufs=1)
        nc.vector.memset(ones65f, 1.0)
        ones65 = small.tile([65, 64], f32r, bufs=1)
        nc.vector.tensor_copy(ones65, ones65f)

        # ---- load + round weights and x^T ----
        with tc.tile_pool(name="stage", bufs=3) as stage, tc.tile_pool(
            name="xtpool", bufs=1
        ) as xtpool:
            wts = []
            for name, src in (("wq", wqT), ("wk", wkT), ("wv", wvT)):
                w_r = xtpool.tile([128, KT, DLOC], f32r, name=f"{name}_r", tag=name)
                srcv = src.rearrange("(k p) m -> p k m", p=128)
                for k in range(KT):
                    st_t = stage.tile([128, 2048], f32, tag="stage", name="st_w")
                    nc.sync.dma_start(out=st_t[:, 0:DLOC], in_=srcv[:, k])
                    nc.vector.tensor_copy(w_r[:, k], st_t[:, 0:DLOC])
                wts.append(w_r)
            wq_r, wk_r, wv_r = wts

            # woT [256, 1024] -> [64 part, HPC, 1024] (head on free axis)
            wo_r = wpool.tile([64, HPC, D], f32r)
            wov = woT.rearrange("(h c) e -> c h e", c=64)
            for h in range(HPC):
                st_t = stage.tile([128, 2048], f32, tag="stage", name="st_wo")
                nc.sync.dma_start(out=st_t[0:64, 0:D], in_=wov[:, h])
                nc.vector.tensor_copy(wo_r[:, h], st_t[0:64, 0:D])

            xt_r = xtpool.tile([128, KT, S], f32r)
            xv = xT.rearrange("(k p) s -> p k s", p=128)
            for k in range(KT):
                st_t = stage.tile([128, 2048], f32, tag="stage", name="st_x")
                nc.sync.dma_start(out=st_t, in_=xv[:, k])
                nc.vector.tensor_copy(xt_r[:, k], st_t)

            # ---- projections ----
            # v [s, dloc] with ones column: v_sb [128, st, h, 65]
            v_sb = qkv.tile([128, ST, HPC, 65], f32r)
            for st_i in range(ST):
                pv = ps.tile([128, DLOC], f32, tag="ps", name="pv")
                for k in range(KT):
                    nc.tensor.matmul(
                        pv,
                        lhsT=xt_r[:, k, st_i * 128 : (st_i + 1) * 128],
                        rhs=wv_r[:, k],
                        start=(k == 0),
                        stop=(k == KT - 1),
                    )
                nc.vector.tensor_copy(
                    v_sb[:, st_i, :, 0:64],
                    pv.rearrange("p (h d) -> p h d", h=HPC),
                )
                nc.vector.tensor_copy(v_sb[:, st_i, :, 64], ones_f)

            # qT, kT [128, 2 m-tiles, S]
            qt = qkv.tile([128, 2, S], f32r)
            kt = qkv.tile([128, 2, S], f32r)
            for dst, w_r in ((qt, wq_r), (kt, wk_r)):
                for m in range(2):
                    for half in range(2):
                        pq = ps.tile([128, IB], f32, tag="ps", name="pq")
                        for k in range(KT):
                            for ch in range(NCH):
                                nc.tensor.matmul(
                                    pq[:, ch * 512 : (ch + 1) * 512],
                                    lhsT=w_r[:, k, m * 128 : (m + 1) * 128],
                                    rhs=xt_r[
                                        :,
                                        k,
                                        half * IB
                                        + ch * 512 : half * IB
                                        + (ch + 1) * 512,
                                    ],
                                    start=(k == 0),
                                    stop=(k == KT - 1),
                                )
                        nc.vector.tensor_copy(
                            dst[:, m, half * IB : (half + 1) * IB], pq
                        )

        # ---- phase C/D pools (allocated after staging space is released) ----
        ptp = ctx.enter_context(tc.tile_pool(name="ptp", bufs=3))
        osb = ctx.enter_context(tc.tile_pool(name="osb", bufs=1))
        outsb = ctx.enter_context(tc.tile_pool(name="outsb", bufs=3))
        norm = ctx.enter_context(tc.tile_pool(name="norm", bufs=2))
        # o^T accumulator in SBUF for all heads/i-blocks (read by phase D)
        o_sb = osb.tile([64, HPC, NB, IB], f32r, name="o_sb")

        # ---- attention + output projection ----
        for ib in range(NB):
            for h in range(HPC):
                p0 = (h % 2) * 64
                mi = h // 2
                o_aug = pso.tile([65, IB], f32, tag="pso", name="o_aug")
                for jt in range(ST):
                    ssc = ps.tile([128, IB], f32, tag="ps", name="ssc")
                    for ch in range(NCH):
                        nc.tensor.matmul(
                            ssc[:, ch * 512 : (ch + 1) * 512],
                            lhsT=kt[p0 : p0 + 64, mi, jt * 128 : (jt + 1) * 128],
                            rhs=qt[
                                p0 : p0 + 64,
                                mi,
                                ib * IB + ch * 512 : ib * IB + (ch + 1) * 512,
                            ],
                            start=True,
                            stop=True,
                        )
                    pt = ptp.tile([128, IB], f32r, tag="pt", name="pt")
                    nc.scalar.activation(pt, ssc, EXP, scale=0.125)
                    for ch in range(NCH):
                        nc.tensor.matmul(
                            o_aug[:, ch * 512 : (ch + 1) * 512],
                            lhsT=v_sb[:, jt, h, :],
                            rhs=pt[:, ch * 512 : (ch + 1) * 512],
                            start=(jt == 0),
                            stop=(jt == ST - 1),
                        )
                # normalize: recip of colsum (row 64), broadcast via PE
                rrow = norm.tile([65, IB], f32r, tag="rrow", name="rrow")
                with nc.allow_low_precision(reason="f32r feed for PE broadcast"):
                    nc.vector.reciprocal(rrow[64:65, :], o_aug[64:65, :])
                rb_ps = ps.tile([64, IB], f32, tag="ps", name="rb_ps")
                for ch in range(NCH):
                    nc.tensor.matmul(
                        rb_ps[:, ch * 512 : (ch + 1) * 512],
                        lhsT=ones65[64:65, :],
                        rhs=rrow[64:65, ch * 512 : (ch + 1) * 512],
                        start=True,
                        stop=True,
                    )
                rb_sb = norm.tile([64, IB], f32, tag="rb_sb", name="rb_sb")
                nc.vector.tensor_copy(rb_sb, rb_ps)
                nc.vector.tensor_mul(o_sb[:, h, ib], o_aug[0:64, :], rb_sb)

            # output projection for this i-block
            for it in range(8):
                po = ps.tile([128, D], f32, tag="ps", name="po")
                for h in range(HPC):
                    for ch in range(2):
                        nc.tensor.matmul(
                            po[:, ch * 512 : (ch + 1) * 512],
                            lhsT=o_sb[:, h, ib, it * 128 : (it + 1) * 128],
                            rhs=wo_r[:, h, ch * 512 : (ch + 1) * 512],
                            start=(h == 0),
                            stop=(h == HPC - 1),
                        )
                ot = outsb.tile([128, D], f32, tag="ot", name="ot")
                nc.vector.tensor_copy(ot, po)
                row = ib * IB + it * 128
                nc.sync.dma_start(out=outp[row : row + 128, :], in_=ot)


_PROGRAM = None


def _program():
    global _PROGRAM
    if _PROGRAM is None:
        nc = bacc.Bacc("TRN2", target_bir_lowering=False, debug=False)
        with tile.TileContext(nc) as tc:
            _emit(tc, nc)
        nc.compile()
        _PROGRAM = nc
    return _PROGRAM


def kernel(x, e, wq, wk, wv, wo, **_unused):
    x = np.asarray(x, dtype=np.float32)
    wq = np.asarray(wq, dtype=np.float32)
    wk = np.asarray(wk, dtype=np.float32)
    wv = np.asarray(wv, dtype=np.float32)
    wo = np.asarray(wo, dtype=np.float32)

    nc = _program()
    in_maps = []
    for c in range(NCORES):
        b, g = divmod(c, GROUPS)
        rows = slice(g * DLOC, (g + 1) * DLOC)
        in_maps.append(
            {
                "xT": np.ascontiguousarray(x[b].T),
                "wqT": np.ascontiguousarray(wq[rows, :].T),
                "wkT": np.ascontiguousarray(wk[rows, :].T),
                "wvT": np.ascontiguousarray(wv[rows, :].T),
                "woT": np.ascontiguousarray(wo[:, rows].T),
            }
        )

    res = run_bass_kernel_spmd(nc, in_maps, list(range(NCORES))).results
    out = np.empty((B, S, D), dtype=np.float32)
    for b in range(B):
        acc = res[b * GROUPS]["outp"].astype(np.float32)
        for g in range(1, GROUPS):
            acc = acc + res[b * GROUPS + g]["outp"]
        out[b] = acc
    return out
